# revision 9
# baseline (speedup 1.0000x reference)
"""Distributed Bass kernel: attention with distance-based positional weights + LayerNorm.

nn_Attention: B=2, S=2048, E=1024, H=16 (d=64), fp32.
  q/k/v = x @ W{q,k,v}.T ; S = q.k^T * E**-0.5 * (|i-j|/S) ; P = softmax(S)
  out = LayerNorm(P @ v)

Sharding: tensor-parallel over heads. 8 cores x 2 heads (128 features each).
Each core computes Q/K/V projections for its 2 heads from the full x,
runs attention, normalizes by the softmax denominator (appended as a
ones-column to V so the PV matmul produces row sums for free), then an
AllToAll re-shards from feature-blocks to token-blocks and each core
LayerNorms its 512-token slice.

Distance-weight trick: A = c*(q-k)*S*sign(q-k) with c = 1/(32*2048).
D = (q-k)*S comes from one 128-contraction matmul with index-augmented
operands (KA = [K^T; k*K^T], QA = [q*Q^T; -Q^T]); sign(q-k) is constant
per region (split at the diagonal k-tile), handled by the ACT exp scale,
with a small per-tile sign mask fixing the 128x128 diagonal block.
All matmuls run as float32r (full PE rate at N>=512, fp32 storage).
"""

import sys
import numpy as np

for _p in ("/opt/trn_rl_repo", "/root/.axon_site/_ro/trn_rl_repo"):
    if _p not in sys.path:
        sys.path.append(_p)

from concourse import bass, bacc, tile, mybir  # noqa: E402
from concourse import bass_utils  # noqa: E402

dt = mybir.dt
AF = mybir.ActivationFunctionType
ALU = mybir.AluOpType

B, S, E, H = 2, 2048, 1024, 16
D = E // H                      # 64
NCORES = 8
PF = E // NCORES                # 128 features per core (2 heads)
NT = B * S                      # 4096 tokens
TOK = NT // NCORES              # 512 tokens per core for LN
NKT = S // 128                  # 16 k-tiles per batch
NET = E // 128                  # 8 e-tiles (contraction) per projection
STRIPE = 1024                   # q-stripe width in attention inner loop
NST = S // STRIPE               # 2 stripes per batch
CEXP = 1.0 / (32.0 * 2048.0)    # E**-0.5 / S
EPS = 1e-5

F32R = dt.float32r
F32 = dt.float32

_CACHE = {}


def _build():
    nc = bacc.Bacc("TRN2", target_bir_lowering=False, debug=False,
                   num_devices=NCORES)

    # ---- DRAM I/O (all float32r == fp32 bytes; numpy side is float32) ----
    xT = nc.dram_tensor("xT", [E, NT], F32R, kind="ExternalInput").ap()
    wq = nc.dram_tensor("wq", [E, PF], F32R, kind="ExternalInput").ap()
    wk = nc.dram_tensor("wk", [E, PF], F32R, kind="ExternalInput").ap()
    wv = nc.dram_tensor("wv", [E, PF], F32R, kind="ExternalInput").ap()
    qidx_d = nc.dram_tensor("qidx", [128, S], F32R, kind="ExternalInput").ap()
    sgnc_d = nc.dram_tensor("sgnc", [128, 128], F32R, kind="ExternalInput").ap()
    ident_d = nc.dram_tensor("ident", [128, 128], F32R, kind="ExternalInput").ap()
    onespc_d = nc.dram_tensor("onespc", [128, 1], F32R, kind="ExternalInput").ap()
    onesr_d = nc.dram_tensor("onesr", [1, 128], F32R, kind="ExternalInput").ap()
    vones_d = nc.dram_tensor("vones", [128, 16 * 65], F32R, kind="ExternalInput").ap()
    gamma_d = nc.dram_tensor("gamma", [E, 1], F32, kind="ExternalInput").ap()
    beta_d = nc.dram_tensor("beta", [E, 1], F32, kind="ExternalInput").ap()
    out_d = nc.dram_tensor("out", [E, TOK], F32R, kind="ExternalOutput").ap()

    with tile.TileContext(nc) as tc:
        with (
            tc.tile_pool(name="res", bufs=1) as res,
            tc.tile_pool(name="work", bufs=1) as work,
            tc.tile_pool(name="psum", bufs=1, space="PSUM") as psum,
            tc.tile_pool(name="dram", bufs=1, space="DRAM") as dram,
            nc.allow_low_precision(reason="float32r is fp32 storage"),
        ):
            # ---------- resident constants ----------
            wq_sb = res.tile([128, NET * 128], F32R, name="wq_sb")
            wk_sb = res.tile([128, NET * 128], F32R, name="wk_sb")
            wv_sb = res.tile([128, NET * 128], F32R, name="wv_sb")
            for kt in range(NET):
                sl = slice(kt * 128, (kt + 1) * 128)
                nc.sync.dma_start(wq_sb[:, sl], wq[sl, :])
                nc.sync.dma_start(wk_sb[:, sl], wk[sl, :])
                nc.sync.dma_start(wv_sb[:, sl], wv[sl, :])
            qidx = res.tile([128, S], F32R, name="qidx")
            nc.sync.dma_start(qidx[:], qidx_d[:])
            sgnc = res.tile([128, 128], F32R, name="sgnc")
            nc.sync.dma_start(sgnc[:], sgnc_d[:])
            ident = res.tile([128, 128], F32R, name="ident")
            nc.sync.dma_start(ident[:], ident_d[:])
            gamma_sb = res.tile([128, NET], F32, name="gamma_sb")
            beta_sb = res.tile([128, NET], F32, name="beta_sb")
            for kt in range(NET):
                nc.sync.dma_start(gamma_sb[:, kt:kt + 1],
                                  gamma_d[kt * 128:(kt + 1) * 128, :])
                nc.sync.dma_start(beta_sb[:, kt:kt + 1],
                                  beta_d[kt * 128:(kt + 1) * 128, :])
            ones_col = res.tile([128, 1], F32R, name="ones_col")
            nc.sync.dma_start(ones_col[:], onespc_d[:])
            ones_row = res.tile([1, 128], F32R, name="ones_row")
            nc.sync.dma_start(ones_row[:], onesr_d[:])
            # normalized attention output, feature-major [128, 4096]
            outT_sb = res.tile([128, NT], F32R, name="outT_sb")

            for b in range(B):
                bsl = slice(b * S, (b + 1) * S)
                # ---------- load x^T for this batch ----------
                xt = work.tile([128, NET * S], F32R, tag="xt", name=f"xt{b}")
                for kt in range(NET):
                    nc.sync.dma_start(
                        xt[:, kt * S:(kt + 1) * S],
                        xT[kt * 128:(kt + 1) * 128, bsl])

                # ---------- Q/K projections with augmentation ----------
                qa = [work.tile([128, S], F32R, tag=f"qa{h}", name=f"qa{h}_{b}")
                      for h in range(2)]
                ka = [work.tile([128, S], F32R, tag=f"ka{h}", name=f"ka{h}_{b}")
                      for h in range(2)]
                for g in range(S // 512):
                    gsl = slice(g * 512, (g + 1) * 512)
                    pq = psum.tile([128, 1024], F32, tag="d", bufs=2, name=f"pq{b}{g}")
                    pk = psum.tile([128, 1024], F32, tag="d", bufs=2, name=f"pk{b}{g}")
                    for kt in range(NET):
                        ksl = slice(kt * 128, (kt + 1) * 128)
                        nc.tensor.matmul(pq[:, 0:512], wq_sb[:, ksl],
                                         xt[:, kt * S + g * 512:
                                            kt * S + (g + 1) * 512],
                                         start=(kt == 0), stop=(kt == NET - 1))
                    for kt in range(NET):
                        ksl = slice(kt * 128, (kt + 1) * 128)
                        nc.tensor.matmul(pk[:, 0:512], wk_sb[:, ksl],
                                         xt[:, kt * S + g * 512:
                                            kt * S + (g + 1) * 512],
                                         start=(kt == 0), stop=(kt == NET - 1))
                    for h in range(2):
                        hs = slice(h * 64, h * 64 + 64)
                        # QA top: qidx * Q^T ; QA bottom: -Q^T
                        nc.vector.tensor_tensor(qa[h][0:64, gsl],
                                                pq[hs, 0:512],
                                                qidx[0:64, gsl], ALU.mult)
                        nc.scalar.activation(qa[h][64:128, gsl],
                                             pq[hs, 0:512], AF.Copy,
                                             scale=-1.0)
                        # KA top: K^T ; KA bottom: kidx * K^T
                        nc.scalar.activation(ka[h][0:64, gsl],
                                             pk[hs, 0:512], AF.Copy)
                        nc.vector.tensor_tensor(ka[h][64:128, gsl],
                                                pk[hs, 0:512],
                                                qidx[0:64, gsl], ALU.mult)

                # ---------- V projection (feature-major) + transpose ----------
                vt = work.tile([128, S], F32R, tag="vt", name=f"vt{b}")
                for g in range(S // 512):
                    pv = psum.tile([128, 1024], F32, tag="d", bufs=2, name=f"pv{b}{g}")
                    for kt in range(NET):
                        ksl = slice(kt * 128, (kt + 1) * 128)
                        nc.tensor.matmul(pv[:, 0:512], wv_sb[:, ksl],
                                         xt[:, kt * S + g * 512:
                                            kt * S + (g + 1) * 512],
                                         start=(kt == 0), stop=(kt == NET - 1))
                    nc.scalar.activation(vt[:, g * 512:(g + 1) * 512],
                                         pv[:, 0:512], AF.Copy)
                # token-major V with a ones column per 65-wide chunk
                vsb = [work.tile([128, NKT * 65], F32R, tag=f"v{h}",
                                 name=f"v{h}_{b}") for h in range(2)]
                for h in range(2):
                    nc.sync.dma_start(vsb[h][:], vones_d[:])
                for c in range(NKT):
                    pt_ps = psum.tile([128, 128], F32R, tag="d", bufs=2,
                                      name=f"ptr{b}{c}")
                    nc.tensor.transpose(pt_ps[:],
                                        vt[:, c * 128:(c + 1) * 128], ident[:])
                    for h in range(2):
                        nc.vector.tensor_copy(
                            vsb[h][:, c * 65:c * 65 + 64],
                            pt_ps[:, h * 64:h * 64 + 64])

                # ---------- attention ----------
                for st in range(NST):
                    po = [psum.tile([65, STRIPE], F32, tag=f"o{h}",
                                    name=f"po{h}_{b}{st}") for h in range(2)]
                    for kt in range(NKT):
                        bound = min(max((kt + 1) * 128 - st * STRIPE, 0), STRIPE)
                        for h in range(2):
                            pd = psum.tile([128, STRIPE], F32, tag="d", bufs=2,
                                           name=f"pd{b}{st}{kt}{h}")
                            for g2 in range(STRIPE // 512):
                                nc.tensor.matmul(
                                    pd[:, g2 * 512:(g2 + 1) * 512],
                                    ka[h][:, kt * 128:(kt + 1) * 128],
                                    qa[h][:, st * STRIPE + g2 * 512:
                                          st * STRIPE + (g2 + 1) * 512],
                                    start=True, stop=True)
                            # diagonal block: fold -sign(q-k) into D so the
                            # whole left region uses exp(-c * D)
                            if kt * 128 >= st * STRIPE and \
                               (kt + 1) * 128 <= (st + 1) * STRIPE:
                                dl = kt * 128 - st * STRIPE
                                nc.vector.tensor_tensor(
                                    pd[:, dl:dl + 128], pd[:, dl:dl + 128],
                                    sgnc[:], ALU.mult)
                            ptile = work.tile([128, STRIPE], F32R, tag="pt",
                                              bufs=3, name=f"pt{b}{st}{kt}{h}")
                            if bound > 0:
                                nc.scalar.activation(ptile[:, 0:bound],
                                                     pd[:, 0:bound], AF.Exp,
                                                     scale=-CEXP)
                            if bound < STRIPE:
                                nc.scalar.activation(ptile[:, bound:STRIPE],
                                                     pd[:, bound:STRIPE],
                                                     AF.Exp, scale=CEXP)
                            for g2 in range(STRIPE // 512):
                                nc.tensor.matmul(
                                    po[h][:, g2 * 512:(g2 + 1) * 512],
                                    vsb[h][:, kt * 65:(kt + 1) * 65],
                                    ptile[:, g2 * 512:(g2 + 1) * 512],
                                    start=(kt == 0), stop=(kt == NKT - 1))
                    # normalize: rows 0:64 / row 64 (softmax denominator)
                    for h in range(2):
                        rec = work.tile([1, STRIPE], F32R, tag="rec",
                                        name=f"rec{b}{st}{h}")
                        nc.vector.reciprocal(rec[:], po[h][64:65, :])
                        pbc = psum.tile([64, STRIPE], F32, tag="d", bufs=2,
                                        name=f"pbc{b}{st}{h}")
                        nc.tensor.matmul(pbc[:, 0:512], ones_row[:, 0:64],
                                         rec[:, 0:512], start=True, stop=True)
                        nc.tensor.matmul(pbc[:, 512:1024], ones_row[:, 0:64],
                                         rec[:, 512:1024], start=True,
                                         stop=True)
                        bc_sb = work.tile([64, STRIPE], F32R, tag="bcsb",
                                          bufs=2, name=f"bc{b}{st}{h}")
                        nc.scalar.activation(bc_sb[:], pbc[:], AF.Copy)
                        nc.vector.tensor_tensor(
                            outT_sb[h * 64:(h + 1) * 64,
                                    b * S + st * STRIPE:
                                    b * S + (st + 1) * STRIPE],
                            po[h][0:64, :], bc_sb[:], ALU.mult)

            # ---------- AllToAll: feature-blocks -> token-blocks ----------
            a2a_in = dram.tile([NCORES * 128, TOK], F32R, name="a2a_in")
            a2a_out = dram.tile([NCORES * 128, TOK], F32R, name="a2a_out")
            for j in range(NCORES):
                nc.sync.dma_start(a2a_in[j * 128:(j + 1) * 128, :],
                                  outT_sb[:, j * TOK:(j + 1) * TOK])
            nc.gpsimd.collective_compute(
                "AllToAll", ALU.bypass,
                replica_groups=[list(range(NCORES))],
                ins=[a2a_in.opt()], outs=[a2a_out.opt()])

            # ---------- LayerNorm over features for 512 tokens ----------
            gt = [work.tile([128, TOK], F32R, tag="gt", bufs=NET,
                            name=f"gt{kt}")
                  for kt in range(NET)]
            for kt in range(NET):
                nc.sync.dma_start(gt[kt][:],
                                  a2a_out[kt * 128:(kt + 1) * 128, :])
            ps_s = psum.tile([1, TOK], F32, tag="o0", name="ps_s")
            ps_q = psum.tile([1, TOK], F32, tag="o1", name="ps_q")
            for kt in range(NET):
                nc.tensor.matmul(ps_s[:], ones_col[:], gt[kt][:],
                                 start=(kt == 0), stop=(kt == NET - 1))
            for kt in range(NET):
                sq = work.tile([128, TOK], F32R, tag="sq", bufs=1,
                               name=f"sq{kt}")
                nc.vector.tensor_tensor(sq[:], gt[kt][:], gt[kt][:], ALU.mult)
                nc.tensor.matmul(ps_q[:], ones_col[:], sq[:],
                                 start=(kt == 0), stop=(kt == NET - 1))
            mean = work.tile([1, TOK], F32R, name="mean")
            nc.vector.tensor_scalar_mul(mean[:], ps_s[:], 1.0 / E)
            m2 = work.tile([1, TOK], F32R, name="m2")
            nc.vector.tensor_tensor(m2[:], mean[:], mean[:], ALU.mult)
            var = work.tile([1, TOK], F32R, name="var")
            nc.vector.tensor_scalar_mul(var[:], ps_q[:], 1.0 / E)
            nc.vector.tensor_tensor(var[:], var[:], m2[:], ALU.subtract)
            eps_t = work.tile([1, 1], F32, name="eps_t")
            nc.vector.memset(eps_t[:], EPS)
            std = work.tile([1, TOK], F32R, name="std")
            nc.scalar.activation(std[:], var[:], AF.Sqrt, bias=eps_t[:])
            rstd = work.tile([1, TOK], F32R, name="rstd")
            nc.vector.reciprocal(rstd[:], std[:])
            nmr = work.tile([1, TOK], F32R, name="nmr")
            nc.vector.tensor_tensor(nmr[:], mean[:], rstd[:], ALU.mult)
            nc.vector.tensor_scalar_mul(nmr[:], nmr[:], -1.0)
            # broadcast rstd / (-mean*rstd) across partitions via ones matmul
            pa = psum.tile([128, TOK], F32, tag="d", bufs=2, name="pa")
            pb = psum.tile([128, TOK], F32, tag="d", bufs=2, name="pb")
            nc.tensor.matmul(pa[:], ones_row[:], rstd[:],
                             start=True, stop=True)
            nc.tensor.matmul(pb[:], ones_row[:], nmr[:],
                             start=True, stop=True)
            for kt in range(NET):
                t1 = work.tile([128, TOK], F32R, tag="t1", bufs=1,
                               name=f"t1{kt}")
                nc.vector.tensor_tensor(t1[:], gt[kt][:], pa[:], ALU.mult)
                nc.vector.tensor_tensor(t1[:], t1[:], pb[:], ALU.add)
                nc.vector.tensor_scalar(t1[:], t1[:],
                                        gamma_sb[:, kt:kt + 1],
                                        beta_sb[:, kt:kt + 1],
                                        ALU.mult, ALU.add)
                nc.sync.dma_start(out_d[kt * 128:(kt + 1) * 128, :], t1[:])

    nc.compile()
    return nc


def _host_inputs(x, Wq, Wk, Wv, ln_gamma, ln_beta):
    xT = np.ascontiguousarray(x.reshape(NT, E).T.astype(np.float32))
    qidx = np.tile(np.arange(S, dtype=np.float32), (128, 1))
    jj = np.arange(128, dtype=np.float32)
    sgnc = -np.sign(jj[None, :] - jj[:, None]).astype(np.float32)
    ident = np.eye(128, dtype=np.float32)
    gamma = np.asarray(ln_gamma, np.float32).reshape(E, 1)
    beta = np.asarray(ln_beta, np.float32).reshape(E, 1)
    in_maps = []
    for c in range(NCORES):
        fsl = slice(c * PF, (c + 1) * PF)
        in_maps.append({
            "xT": xT,
            "wq": np.ascontiguousarray(np.asarray(Wq, np.float32)[fsl, :].T),
            "wk": np.ascontiguousarray(np.asarray(Wk, np.float32)[fsl, :].T),
            "wv": np.ascontiguousarray(np.asarray(Wv, np.float32)[fsl, :].T),
            "qidx": qidx,
            "sgnc": sgnc,
            "ident": ident,
            "onespc": np.ones((128, 1), np.float32),
            "onesr": np.ones((1, 128), np.float32),
            "vones": np.ones((128, 16 * 65), np.float32),
            "gamma": gamma,
            "beta": beta,
        })
    return in_maps


def kernel(x, Wq, Wk, Wv, ln_gamma, ln_beta, _trace=False, _tmpdir=None):
    if "nc" not in _CACHE:
        _CACHE["nc"] = _build()
    nc = _CACHE["nc"]
    in_maps = _host_inputs(x, Wq, Wk, Wv, ln_gamma, ln_beta)
    res = bass_utils.run_bass_kernel_spmd(
        nc, in_maps, core_ids=list(range(NCORES)),
        trace=_trace, tmpdir=_tmpdir)
    _CACHE["last_result"] = res
    outT = np.concatenate([np.asarray(res.results[c]["out"])
                           for c in range(NCORES)], axis=1)
    return np.ascontiguousarray(outT.T).reshape(B, S, E).astype(np.float32)


# revision 11
# speedup vs baseline: 1.0372x; 1.0372x over previous
"""Distributed Bass kernel: attention with distance-based positional weights + LayerNorm.

nn_Attention: B=2, S=2048, E=1024, H=16 (d=64), fp32.
  q/k/v = x @ W{q,k,v}.T ; S = q.k^T * E**-0.5 * (|i-j|/S) ; P = softmax(S)
  out = LayerNorm(P @ v)

Sharding: tensor-parallel over heads. 8 cores x 2 heads (128 features each).
Each core computes Q/K/V projections for its 2 heads from the full x,
runs attention, normalizes by the softmax denominator (appended as a
ones-column to V so the PV matmul produces row sums for free), then an
AllToAll re-shards from feature-blocks to token-blocks and each core
LayerNorms 2x256 tokens (256 per batch, so each batch's AllToAll can
overlap the other batch's compute).

Distance-weight trick: A = c*(q-k)*S*sign(q-k) with c = 1/(32*2048).
D = (q-k)*S comes from one 128-contraction matmul with index-augmented
operands (KA = [K^T; k*K^T], QA = [q*Q^T; -Q^T]); sign(q-k) is constant
per region (split at the diagonal k-tile), handled by the ACT exp scale,
with a small per-tile sign mask fixing the 128x128 diagonal block.
All matmuls run as float32r (full PE rate at N>=512, fp32 storage).
"""

import sys
import numpy as np

for _p in ("/opt/trn_rl_repo", "/root/.axon_site/_ro/trn_rl_repo"):
    if _p not in sys.path:
        sys.path.append(_p)

from concourse import bass, bacc, tile, mybir  # noqa: E402
from concourse import bass_utils  # noqa: E402

dt = mybir.dt
AF = mybir.ActivationFunctionType
ALU = mybir.AluOpType

B, S, E, H = 2, 2048, 1024, 16
D = E // H                      # 64
NCORES = 8
PF = E // NCORES                # 128 features per core (2 heads)
NT = B * S                      # 4096 tokens
HTOK = 256                      # tokens per core per batch for LN
NKT = S // 128                  # 16 k-tiles per batch
NET = E // 128                  # 8 e-tiles (contraction) per projection
STRIPE = 1024                   # q-stripe width in attention inner loop
NST = S // STRIPE               # 2 stripes per batch
CEXP = 1.0 / (32.0 * 2048.0)    # E**-0.5 / S
EPS = 1e-5

F32R = dt.float32r
F32 = dt.float32

_CACHE = {}


def _build():
    nc = bacc.Bacc("TRN2", target_bir_lowering=False, debug=False,
                   num_devices=NCORES)

    # ---- DRAM I/O (float32r == fp32 bytes; numpy side is float32) ----
    xT = nc.dram_tensor("xT", [E, NT], F32R, kind="ExternalInput").ap()
    wq = nc.dram_tensor("wq", [E, PF], F32R, kind="ExternalInput").ap()
    wk = nc.dram_tensor("wk", [E, PF], F32R, kind="ExternalInput").ap()
    wv = nc.dram_tensor("wv", [E, PF], F32R, kind="ExternalInput").ap()
    qidx_d = nc.dram_tensor("qidx", [128, S], F32R, kind="ExternalInput").ap()
    sgnc_d = nc.dram_tensor("sgnc", [128, 128], F32R, kind="ExternalInput").ap()
    ident_d = nc.dram_tensor("ident", [128, 128], F32R, kind="ExternalInput").ap()
    onesr_d = nc.dram_tensor("onesr", [1, 128], F32, kind="ExternalInput").ap()
    onespc_d = nc.dram_tensor("onespc", [128, 1], F32R, kind="ExternalInput").ap()
    vones_d = nc.dram_tensor("vones", [128, 16 * 65], F32R, kind="ExternalInput").ap()
    gamma_d = nc.dram_tensor("gamma", [E, 1], F32, kind="ExternalInput").ap()
    beta_d = nc.dram_tensor("beta", [E, 1], F32, kind="ExternalInput").ap()
    out_d = nc.dram_tensor("out", [E, 2 * HTOK], F32R, kind="ExternalOutput").ap()

    with tile.TileContext(nc) as tc:
        with (
            tc.tile_pool(name="res", bufs=1) as res,
            tc.tile_pool(name="work", bufs=1) as work,
            tc.tile_pool(name="psum", bufs=1, space="PSUM") as psum,
            tc.tile_pool(name="dram", bufs=1, space="DRAM") as dram,
            nc.allow_low_precision(reason="float32r is fp32 storage"),
        ):
            # ---------- resident constants ----------
            wq_sb = res.tile([128, NET * 128], F32R, name="wq_sb")
            wk_sb = res.tile([128, NET * 128], F32R, name="wk_sb")
            wv_sb = res.tile([128, NET * 128], F32R, name="wv_sb")
            for kt in range(NET):
                sl = slice(kt * 128, (kt + 1) * 128)
                nc.sync.dma_start(wq_sb[:, sl], wq[sl, :])
                nc.sync.dma_start(wk_sb[:, sl], wk[sl, :])
                nc.sync.dma_start(wv_sb[:, sl], wv[sl, :])
            qidx = res.tile([128, S], F32R, name="qidx")
            nc.sync.dma_start(qidx[:], qidx_d[:])
            sgnc = res.tile([128, 128], F32R, name="sgnc")
            nc.sync.dma_start(sgnc[:], sgnc_d[:])
            ident = res.tile([128, 128], F32R, name="ident")
            nc.sync.dma_start(ident[:], ident_d[:])
            gamma_sb = res.tile([128, NET], F32, name="gamma_sb")
            beta_sb = res.tile([128, NET], F32, name="beta_sb")
            for kt in range(NET):
                nc.sync.dma_start(gamma_sb[:, kt:kt + 1],
                                  gamma_d[kt * 128:(kt + 1) * 128, :])
                nc.sync.dma_start(beta_sb[:, kt:kt + 1],
                                  beta_d[kt * 128:(kt + 1) * 128, :])
            ones_col = res.tile([128, 1], F32R, name="ones_col")
            nc.sync.dma_start(ones_col[:], onespc_d[:])
            ones_row = res.tile([1, 128], F32, name="ones_row")
            nc.sync.dma_start(ones_row[:], onesr_d[:])
            # normalized attention output, feature-major [128, 4096]
            outT_sb = res.tile([128, NT], F32R, name="outT_sb")

            a2a_in = [dram.tile([NCORES * 128, HTOK], F32R, name=f"a2a_in{b}")
                      for b in range(B)]
            a2a_out = [dram.tile([NCORES * 128, HTOK], F32R, name=f"a2a_out{b}")
                       for b in range(B)]

            def layer_norm(b):
                """LN over features for this core's HTOK tokens of batch b."""
                gt = [work.tile([128, HTOK], F32R, tag=f"gt{kt}",
                                name=f"gt{b}_{kt}") for kt in range(NET)]
                for kt in range(NET):
                    nc.sync.dma_start(gt[kt][:],
                                      a2a_out[b][kt * 128:(kt + 1) * 128, :])
                ps_s = psum.tile([1, HTOK], F32, tag="o0", name=f"ps_s{b}")
                ps_q = psum.tile([1, HTOK], F32, tag="o1", name=f"ps_q{b}")
                for kt in range(NET):
                    nc.tensor.matmul(ps_s[:], ones_col[:], gt[kt][:],
                                     start=(kt == 0), stop=(kt == NET - 1))
                for kt in range(NET):
                    sq = work.tile([128, HTOK], F32R, tag="sq", bufs=2,
                                   name=f"sq{b}_{kt}")
                    nc.vector.tensor_tensor(sq[:], gt[kt][:], gt[kt][:],
                                            ALU.mult)
                    nc.tensor.matmul(ps_q[:], ones_col[:], sq[:],
                                     start=(kt == 0), stop=(kt == NET - 1))
                mean = work.tile([1, HTOK], F32, tag="ln1", name=f"mean{b}")
                nc.vector.tensor_scalar_mul(mean[:], ps_s[:], 1.0 / E)
                m2 = work.tile([1, HTOK], F32, tag="ln2", name=f"m2{b}")
                nc.vector.tensor_tensor(m2[:], mean[:], mean[:], ALU.mult)
                var = work.tile([1, HTOK], F32, tag="ln3", name=f"var{b}")
                nc.vector.tensor_scalar_mul(var[:], ps_q[:], 1.0 / E)
                nc.vector.tensor_tensor(var[:], var[:], m2[:], ALU.subtract)
                eps_t = work.tile([1, 1], F32, tag="ln4", name=f"eps{b}")
                nc.vector.memset(eps_t[:], EPS)
                std = work.tile([1, HTOK], F32, tag="ln5", name=f"std{b}")
                nc.scalar.activation(std[:], var[:], AF.Sqrt, bias=eps_t[:])
                rstd = work.tile([1, HTOK], F32, tag="ln6", name=f"rstd{b}")
                nc.vector.reciprocal_approx_fast(rstd[:], std[:])
                nmr = work.tile([1, HTOK], F32, tag="ln7", name=f"nmr{b}")
                nc.vector.tensor_tensor(nmr[:], mean[:], rstd[:], ALU.mult)
                nc.vector.tensor_scalar_mul(nmr[:], nmr[:], -1.0)
                pa = psum.tile([128, HTOK], F32, tag="d", bufs=2,
                               name=f"pa{b}")
                pb = psum.tile([128, HTOK], F32, tag="d", bufs=2,
                               name=f"pb{b}")
                nc.tensor.matmul(pa[:], ones_row[:], rstd[:],
                                 start=True, stop=True)
                nc.tensor.matmul(pb[:], ones_row[:], nmr[:],
                                 start=True, stop=True)
                for kt in range(NET):
                    t1 = work.tile([128, HTOK], F32R, tag="t1", bufs=2,
                                   name=f"t1{b}_{kt}")
                    nc.vector.tensor_tensor(t1[:], gt[kt][:], pa[:], ALU.mult)
                    nc.vector.tensor_tensor(t1[:], t1[:], pb[:], ALU.add)
                    nc.vector.tensor_scalar(t1[:], t1[:],
                                            gamma_sb[:, kt:kt + 1],
                                            beta_sb[:, kt:kt + 1],
                                            ALU.mult, ALU.add)
                    nc.sync.dma_start(
                        out_d[kt * 128:(kt + 1) * 128,
                              b * HTOK:(b + 1) * HTOK], t1[:])

            for b in range(B):
                bsl = slice(b * S, (b + 1) * S)
                # ---------- load x^T for this batch ----------
                xt = work.tile([128, NET * S], F32R, tag="xt", name=f"xt{b}")
                for kt in range(NET):
                    nc.sync.dma_start(
                        xt[:, kt * S:(kt + 1) * S],
                        xT[kt * 128:(kt + 1) * 128, bsl])

                # ---------- Q/K projections with augmentation ----------
                qa = [work.tile([128, S], F32R, tag=f"qa{h}", name=f"qa{h}_{b}")
                      for h in range(2)]
                ka = [work.tile([128, S], F32R, tag=f"ka{h}", name=f"ka{h}_{b}")
                      for h in range(2)]
                for g in range(S // 512):
                    gsl = slice(g * 512, (g + 1) * 512)
                    pq = psum.tile([128, 1024], F32, tag="d", bufs=2,
                                   name=f"pq{b}{g}")
                    pk = psum.tile([128, 1024], F32, tag="d", bufs=2,
                                   name=f"pk{b}{g}")
                    for kt in range(NET):
                        ksl = slice(kt * 128, (kt + 1) * 128)
                        nc.tensor.matmul(pq[:, 0:512], wq_sb[:, ksl],
                                         xt[:, kt * S + g * 512:
                                            kt * S + (g + 1) * 512],
                                         start=(kt == 0), stop=(kt == NET - 1))
                    for kt in range(NET):
                        ksl = slice(kt * 128, (kt + 1) * 128)
                        nc.tensor.matmul(pk[:, 0:512], wk_sb[:, ksl],
                                         xt[:, kt * S + g * 512:
                                            kt * S + (g + 1) * 512],
                                         start=(kt == 0), stop=(kt == NET - 1))
                    for h in range(2):
                        hs = slice(h * 64, h * 64 + 64)
                        # QA top: qidx * Q^T ; QA bottom: -Q^T
                        nc.vector.tensor_tensor(qa[h][0:64, gsl],
                                                pq[hs, 0:512],
                                                qidx[0:64, gsl], ALU.mult)
                        nc.vector.tensor_scalar_mul(qa[h][64:128, gsl],
                                                    pq[hs, 0:512], -1.0)
                        # KA top: K^T ; KA bottom: kidx * K^T
                        nc.scalar.activation(ka[h][0:64, gsl],
                                             pk[hs, 0:512], AF.Copy)
                        nc.vector.tensor_tensor(ka[h][64:128, gsl],
                                                pk[hs, 0:512],
                                                qidx[0:64, gsl], ALU.mult)

                # ---------- V projection (feature-major) + transpose ----------
                vt = work.tile([128, S], F32R, tag="vt", name=f"vt{b}")
                for g in range(S // 512):
                    pv = psum.tile([128, 1024], F32, tag="d", bufs=2,
                                   name=f"pv{b}{g}")
                    for kt in range(NET):
                        ksl = slice(kt * 128, (kt + 1) * 128)
                        nc.tensor.matmul(pv[:, 0:512], wv_sb[:, ksl],
                                         xt[:, kt * S + g * 512:
                                            kt * S + (g + 1) * 512],
                                         start=(kt == 0), stop=(kt == NET - 1))
                    nc.scalar.activation(vt[:, g * 512:(g + 1) * 512],
                                         pv[:, 0:512], AF.Copy)
                # token-major V with a ones column per 65-wide chunk
                vsb = [work.tile([128, NKT * 65], F32R, tag=f"v{h}",
                                 name=f"v{h}_{b}") for h in range(2)]
                for h in range(2):
                    nc.sync.dma_start(vsb[h][:], vones_d[:])
                for c in range(NKT):
                    pt_ps = psum.tile([128, 128], F32R, tag="d", bufs=2,
                                      name=f"ptr{b}{c}")
                    nc.tensor.transpose(pt_ps[:],
                                        vt[:, c * 128:(c + 1) * 128], ident[:])
                    for h in range(2):
                        nc.vector.tensor_copy(
                            vsb[h][:, c * 65:c * 65 + 64],
                            pt_ps[:, h * 64:h * 64 + 64])

                # ---------- attention ----------
                for st in range(NST):
                    po = [psum.tile([65, STRIPE], F32, tag=f"o{h}",
                                    name=f"po{h}_{b}{st}") for h in range(2)]
                    for kt in range(NKT):
                        bound = min(max((kt + 1) * 128 - st * STRIPE, 0), STRIPE)
                        for h in range(2):
                            pd = psum.tile([128, STRIPE], F32, tag="d", bufs=2,
                                           name=f"pd{b}{st}{kt}{h}")
                            for g2 in range(STRIPE // 512):
                                nc.tensor.matmul(
                                    pd[:, g2 * 512:(g2 + 1) * 512],
                                    ka[h][:, kt * 128:(kt + 1) * 128],
                                    qa[h][:, st * STRIPE + g2 * 512:
                                          st * STRIPE + (g2 + 1) * 512],
                                    start=True, stop=True)
                            # diagonal block: fold -sign(q-k) into D so the
                            # whole left region uses exp(-c * D)
                            if kt * 128 >= st * STRIPE and \
                               (kt + 1) * 128 <= (st + 1) * STRIPE:
                                dl = kt * 128 - st * STRIPE
                                nc.vector.tensor_tensor(
                                    pd[:, dl:dl + 128], pd[:, dl:dl + 128],
                                    sgnc[:], ALU.mult)
                            ptile = work.tile([128, STRIPE], F32R, tag="pt",
                                              bufs=3, name=f"pt{b}{st}{kt}{h}")
                            if bound > 0:
                                nc.scalar.activation(ptile[:, 0:bound],
                                                     pd[:, 0:bound], AF.Exp,
                                                     scale=-CEXP)
                            if bound < STRIPE:
                                nc.scalar.activation(ptile[:, bound:STRIPE],
                                                     pd[:, bound:STRIPE],
                                                     AF.Exp, scale=CEXP)
                            for g2 in range(STRIPE // 512):
                                nc.tensor.matmul(
                                    po[h][:, g2 * 512:(g2 + 1) * 512],
                                    vsb[h][:, kt * 65:(kt + 1) * 65],
                                    ptile[:, g2 * 512:(g2 + 1) * 512],
                                    start=(kt == 0), stop=(kt == NKT - 1))
                    # normalize: rows 0:64 / row 64 (softmax denominator)
                    for h in range(2):
                        den = work.tile([1, STRIPE], F32, tag="den",
                                        name=f"den{b}{st}{h}")
                        nc.vector.tensor_copy(den[:], po[h][64:65, :])
                        rec = work.tile([1, STRIPE], F32, tag="rec",
                                        name=f"rec{b}{st}{h}")
                        nc.vector.reciprocal_approx_fast(rec[:], den[:])
                        pbc = psum.tile([64, STRIPE], F32, tag="d", bufs=2,
                                        name=f"pbc{b}{st}{h}")
                        nc.tensor.matmul(pbc[:, 0:512], ones_row[:, 0:64],
                                         rec[:, 0:512], start=True, stop=True)
                        nc.tensor.matmul(pbc[:, 512:1024], ones_row[:, 0:64],
                                         rec[:, 512:1024], start=True,
                                         stop=True)
                        bc_sb = work.tile([64, STRIPE], F32R, tag="bcsb",
                                          bufs=2, name=f"bc{b}{st}{h}")
                        nc.scalar.activation(bc_sb[:], pbc[:], AF.Copy)
                        nc.vector.tensor_tensor(
                            outT_sb[h * 64:(h + 1) * 64,
                                    b * S + st * STRIPE:
                                    b * S + (st + 1) * STRIPE],
                            po[h][0:64, :], bc_sb[:], ALU.mult)

                # ---- AllToAll for this batch: core c gets batch-b tokens
                # [256c, 256(c+1)); overlaps the next batch's compute ----
                for j in range(NCORES):
                    nc.sync.dma_start(
                        a2a_in[b][j * 128:(j + 1) * 128, :],
                        outT_sb[:, b * S + j * HTOK:b * S + (j + 1) * HTOK])
                nc.gpsimd.collective_compute(
                    "AllToAll", ALU.bypass,
                    replica_groups=[list(range(NCORES))],
                    ins=[a2a_in[b].opt()], outs=[a2a_out[b].opt()])
                layer_norm(b)

    nc.compile()
    return nc


def _host_inputs(x, Wq, Wk, Wv, ln_gamma, ln_beta):
    xT = np.ascontiguousarray(x.reshape(NT, E).T.astype(np.float32))
    qidx = np.tile(np.arange(S, dtype=np.float32), (128, 1))
    jj = np.arange(128, dtype=np.float32)
    sgnc = -np.sign(jj[None, :] - jj[:, None]).astype(np.float32)
    ident = np.eye(128, dtype=np.float32)
    gamma = np.asarray(ln_gamma, np.float32).reshape(E, 1)
    beta = np.asarray(ln_beta, np.float32).reshape(E, 1)
    in_maps = []
    for c in range(NCORES):
        fsl = slice(c * PF, (c + 1) * PF)
        in_maps.append({
            "xT": xT,
            "wq": np.ascontiguousarray(np.asarray(Wq, np.float32)[fsl, :].T),
            "wk": np.ascontiguousarray(np.asarray(Wk, np.float32)[fsl, :].T),
            "wv": np.ascontiguousarray(np.asarray(Wv, np.float32)[fsl, :].T),
            "qidx": qidx,
            "sgnc": sgnc,
            "ident": ident,
            "onesr": np.ones((1, 128), np.float32),
            "onespc": np.ones((128, 1), np.float32),
            "vones": np.ones((128, 16 * 65), np.float32),
            "gamma": gamma,
            "beta": beta,
        })
    return in_maps


def kernel(x, Wq, Wk, Wv, ln_gamma, ln_beta, _trace=False, _tmpdir=None):
    if "nc" not in _CACHE:
        _CACHE["nc"] = _build()
    nc = _CACHE["nc"]
    in_maps = _host_inputs(x, Wq, Wk, Wv, ln_gamma, ln_beta)
    res = bass_utils.run_bass_kernel_spmd(
        nc, in_maps, core_ids=list(range(NCORES)),
        trace=_trace, tmpdir=_tmpdir)
    _CACHE["last_result"] = res
    # out[c] is [E, 2*HTOK]: cols 0:256 = batch-0 tokens [256c, 256(c+1)),
    # cols 256:512 = batch-1 tokens [256c, 256(c+1)) of batch 1.
    outT = np.empty((E, NT), np.float32)
    for c in range(NCORES):
        o = np.asarray(res.results[c]["out"])
        outT[:, c * HTOK:(c + 1) * HTOK] = o[:, 0:HTOK]
        outT[:, S + c * HTOK:S + (c + 1) * HTOK] = o[:, HTOK:]
    return np.ascontiguousarray(outT.T).reshape(B, S, E).astype(np.float32)
